# revision 1
# baseline (speedup 1.0000x reference)
"""Trainium2 Bass kernel for nn_Aggregator_32959579030024.

Computes out[n, d] = curr_emb[n, 0, d] + sum_k alpha[n, k, 0] * msg[n, k, d]
for N=100000, K=32, D=128 (fp32), sharded over 8 NeuronCores on the node dim.

Per-core layout (12500 nodes/shard, 128-node tiles):
  - msg rows for a tile are loaded so SBUF partition p holds msg row 128*g + p
    of the tile (g = 4-node group index, 32 groups/tile). For each group, the
    128 partitions are the (node-in-group m, neighbor k) rows of 4 nodes.
  - A block-diagonal alpha tile [128, 4] per group (alpha[4g+m, k] at
    partition 32m+k, column m) is the moving operand of an fp32 matmul with
    the msg slice [128, 128] as the (self-loading) stationary operand:
        psum[d, m] = sum_{m,k} msg[(m,k), d] * alphadiag[(m,k), m]
                   = sum_k alpha[node, k] * msg[node, k, d]
    PSUM accumulates the tile as [d, node] (transposed).
  - DVE adds the host-transposed curr (currT [D, NS]) during PSUM evacuation;
    the d-major result is DMA'd out and the host transposes it back.
"""

import numpy as np

N, K, D = 100000, 32, 128
CORES = 8
NS = N // CORES              # 12500 nodes per shard
TILE_N = 128                 # nodes per tile
NT_FULL = NS // TILE_N       # 97 full tiles
REM = NS - NT_FULL * TILE_N  # 84-node remainder tile
NT = NT_FULL + (1 if REM else 0)

_cache = {}


def build_program(ns=NS, tile_n=TILE_N, msg_bufs=4):
    import concourse.bacc as bacc
    import concourse.mybir as mybir
    import concourse.tile as tile

    nt_full = ns // tile_n
    rem = ns - nt_full * tile_n
    nt = nt_full + (1 if rem else 0)
    assert tile_n % 4 == 0 and rem % 4 == 0

    nc = bacc.Bacc("TRN2", target_bir_lowering=False, debug=False)
    f32 = mybir.dt.float32
    msg = nc.dram_tensor("msg", [ns * K, D], f32, kind="ExternalInput")
    alpha = nc.dram_tensor("alpha", [ns, K], f32, kind="ExternalInput")
    currT = nc.dram_tensor("currT", [D, ns], f32, kind="ExternalInput")
    out = nc.dram_tensor("out", [nt, D, tile_n], f32, kind="ExternalOutput")

    with tile.TileContext(nc) as tc:
        with (
            tc.tile_pool(name="msgp", bufs=msg_bufs) as msgp,
            tc.tile_pool(name="smallp", bufs=msg_bufs) as smallp,
            tc.tile_pool(name="outp", bufs=msg_bufs) as outp,
            tc.tile_pool(name="psump", bufs=4, space="PSUM") as psump,
        ):
            for t in range(nt):
                nn = tile_n if t < nt_full else rem
                ng = nn // 4
                nb = t * tile_n

                msg_t = msgp.tile([128, ng, D], f32, tag="msg")
                nc.sync.dma_start(
                    msg_t[:],
                    msg[nb * K:(nb + nn) * K, :].rearrange(
                        "(g p) d -> p g d", p=128
                    ),
                )

                al_t = smallp.tile([128, ng, 4], f32, tag="alpha")
                nc.vector.memset(al_t[:], 0.0)
                asrc = alpha[nb:nb + nn, :].rearrange("(j f) k -> f k j", f=4)
                for r in range(4):
                    nc.sync.dma_start(al_t[32 * r:32 * (r + 1), :, r], asrc[r])

                cur_t = smallp.tile([128, tile_n], f32, tag="curr")
                nc.sync.dma_start(cur_t[:, :nn], currT[:, nb:nb + nn])

                ps = psump.tile([128, tile_n], f32, tag="ps")
                for g in range(ng):
                    nc.tensor.matmul(
                        ps[:, 4 * g:4 * g + 4],
                        msg_t[:, g, :],
                        al_t[:, g, :],
                        start=True,
                        stop=True,
                    )

                ot = outp.tile([128, tile_n], f32, tag="out")
                nc.vector.tensor_add(ot[:, :nn], ps[:, :nn], cur_t[:, :nn])
                nc.sync.dma_start(out[t, :, :nn], ot[:, :nn])

    nc.compile()
    return nc


def make_in_maps(curr_emb, alpha, msg, ns=NS):
    curr_emb = np.asarray(curr_emb, dtype=np.float32)
    alpha = np.asarray(alpha, dtype=np.float32)
    msg = np.asarray(msg, dtype=np.float32)
    n = curr_emb.shape[0]
    cores = n // ns
    in_maps = []
    for c in range(cores):
        sl = slice(c * ns, (c + 1) * ns)
        in_maps.append({
            "msg": np.ascontiguousarray(msg[sl].reshape(ns * K, D)),
            "alpha": np.ascontiguousarray(alpha[sl, :, 0]),
            "currT": np.ascontiguousarray(curr_emb[sl, 0, :].T),
        })
    return in_maps


def gather_out(per_core_outs, ns=NS, tile_n=TILE_N):
    shards = []
    for o in per_core_outs:
        nt = o.shape[0]
        # [nt, D, tile_n] -> [nt, tile_n, D] -> [nt*tile_n, D] -> [ns, D]
        shards.append(o.transpose(0, 2, 1).reshape(nt * tile_n, D)[:ns])
    return np.concatenate(shards, axis=0)


def kernel(curr_emb, alpha, msg):
    from concourse.bass_utils import run_bass_kernel_spmd

    if "nc" not in _cache:
        _cache["nc"] = build_program()
    nc = _cache["nc"]
    in_maps = make_in_maps(curr_emb, alpha, msg)
    res = run_bass_kernel_spmd(nc, in_maps, list(range(CORES)))
    return gather_out([res.results[c]["out"] for c in range(CORES)])


# revision 2
# speedup vs baseline: 1.0355x; 1.0355x over previous
"""Trainium2 Bass kernel for nn_Aggregator_32959579030024.

Computes out[n, d] = curr_emb[n, 0, d] + sum_k alpha[n, k, 0] * msg[n, k, d]
for N=100000, K=32, D=128 (fp32), sharded over 8 NeuronCores on the node dim.

Math: per 128-node tile, SBUF partition p holds msg row 128*g + p of the tile
(g = 4-node group, 32 groups/tile); each group's 128 partitions are the
(node-in-group m, neighbor k) rows of 4 nodes. A block-diagonal alpha tile
[128, 4] per group (alpha[4g+m, k] at partition 32m+k, column m) is the moving
operand of a matmul whose stationary operand is the msg slice [128, 128]:

    psum[d, m] += sum_{p=(m,k)} msg[(m,k), d] * alphadiag[(m,k), m]
               =  sum_k alpha[node, k] * msg[node, k, d]

PSUM holds the tile transposed as [d, node]. DVE adds host-transposed curr
(currT [D, NS]) during PSUM evacuation; the d-major result is DMA'd out and
the host transposes it back.

Precision/perf: fp32 matmuls on trn2 stream weights at ~38 Gelem/s (2 full
HI/LO weight-load passes) — PE-bound. Instead the host splits msg exactly
into bf16 hi + bf16 lo (same DMA bytes: 2x2B) and alpha into a + b, and each
group runs 3 bf16 matmuls accumulating in fp32 PSUM:
    m*alpha ~= h*a + h*b + l*a      (dropped l*b term ~2^-18 relative)
bf16 weight loads use the fast-weight-load path, cutting PE time ~4x.
"""

import numpy as np

N, K, D = 100000, 32, 128
CORES = 8
NS = N // CORES              # 12500 nodes per shard
TILE_N = 128                 # nodes per tile
NT_FULL = NS // TILE_N       # 97 full tiles
REM = NS - NT_FULL * TILE_N  # 84-node remainder tile
NT = NT_FULL + (1 if REM else 0)

_cache = {}


def build_program(ns=NS, tile_n=TILE_N, msg_bufs=4):
    import concourse.bacc as bacc
    import concourse.mybir as mybir
    import concourse.tile as tile

    nt_full = ns // tile_n
    rem = ns - nt_full * tile_n
    nt = nt_full + (1 if rem else 0)
    assert tile_n % 4 == 0 and rem % 4 == 0

    nc = bacc.Bacc("TRN2", target_bir_lowering=False, debug=False)
    f32 = mybir.dt.float32
    bf16 = mybir.dt.bfloat16
    # msg2 rows pack [hi(D) | lo(D)] bf16 per original msg row.
    msg2 = nc.dram_tensor("msg2", [ns * K, 2, D], bf16, kind="ExternalInput")
    al_hi = nc.dram_tensor("al_hi", [ns, K], bf16, kind="ExternalInput")
    al_lo = nc.dram_tensor("al_lo", [ns, K], bf16, kind="ExternalInput")
    currT = nc.dram_tensor("currT", [D, ns], f32, kind="ExternalInput")
    out = nc.dram_tensor("out", [nt, D, tile_n], f32, kind="ExternalOutput")

    with tile.TileContext(nc) as tc:
        with (
            tc.tile_pool(name="msgp", bufs=msg_bufs) as msgp,
            tc.tile_pool(name="smallp", bufs=msg_bufs) as smallp,
            tc.tile_pool(name="outp", bufs=msg_bufs) as outp,
            tc.tile_pool(name="psump", bufs=4, space="PSUM") as psump,
        ):
            for t in range(nt):
                nn = tile_n if t < nt_full else rem
                ng = nn // 4
                nb = t * tile_n

                msg_t = msgp.tile([128, ng, 2, D], bf16, tag="msg")
                nc.sync.dma_start(
                    msg_t[:],
                    msg2[nb * K:(nb + nn) * K, :, :].rearrange(
                        "(g p) two d -> p g two d", p=128
                    ),
                )

                ah_t = smallp.tile([128, ng, 4], bf16, tag="ah")
                al_t = smallp.tile([128, ng, 4], bf16, tag="al")
                nc.vector.memset(ah_t[:], 0.0)
                nc.vector.memset(al_t[:], 0.0)
                ah_src = al_hi[nb:nb + nn, :].rearrange("(j f) k -> f k j", f=4)
                al_src = al_lo[nb:nb + nn, :].rearrange("(j f) k -> f k j", f=4)
                for r in range(4):
                    nc.sync.dma_start(ah_t[32 * r:32 * (r + 1), :, r], ah_src[r])
                    nc.sync.dma_start(al_t[32 * r:32 * (r + 1), :, r], al_src[r])

                cur_t = smallp.tile([128, tile_n], f32, tag="curr")
                nc.sync.dma_start(cur_t[:, :nn], currT[:, nb:nb + nn])

                ps = psump.tile([128, tile_n], f32, tag="ps")
                for g in range(ng):
                    o = ps[:, 4 * g:4 * g + 4]
                    h = msg_t[:, g, 0, :]
                    lo = msg_t[:, g, 1, :]
                    a = ah_t[:, g, :]
                    b = al_t[:, g, :]
                    nc.tensor.matmul(o, h, a, start=True, stop=False)
                    nc.tensor.matmul(o, h, b, start=False, stop=False)
                    nc.tensor.matmul(o, lo, a, start=False, stop=True)

                ot = outp.tile([128, tile_n], f32, tag="out")
                nc.vector.tensor_add(ot[:, :nn], ps[:, :nn], cur_t[:, :nn])
                nc.sync.dma_start(out[t, :, :nn], ot[:, :nn])

    nc.compile()
    return nc


def _split_bf16(x):
    import ml_dtypes

    hi = x.astype(ml_dtypes.bfloat16)
    lo = (x - hi.astype(np.float32)).astype(ml_dtypes.bfloat16)
    return hi, lo


def make_in_maps(curr_emb, alpha, msg, ns=NS):
    curr_emb = np.asarray(curr_emb, dtype=np.float32)
    alpha = np.asarray(alpha, dtype=np.float32)
    msg = np.asarray(msg, dtype=np.float32)
    n = curr_emb.shape[0]
    cores = n // ns
    in_maps = []
    for c in range(cores):
        sl = slice(c * ns, (c + 1) * ns)
        m_hi, m_lo = _split_bf16(msg[sl].reshape(ns * K, D))
        msg2 = np.stack([m_hi, m_lo], axis=1)  # [ns*K, 2, D]
        a_hi, a_lo = _split_bf16(alpha[sl, :, 0])
        in_maps.append({
            "msg2": np.ascontiguousarray(msg2),
            "al_hi": np.ascontiguousarray(a_hi),
            "al_lo": np.ascontiguousarray(a_lo),
            "currT": np.ascontiguousarray(curr_emb[sl, 0, :].T),
        })
    return in_maps


def gather_out(per_core_outs, ns=NS, tile_n=TILE_N):
    shards = []
    for o in per_core_outs:
        nt = o.shape[0]
        # [nt, D, tile_n] -> [nt, tile_n, D] -> [nt*tile_n, D] -> [ns, D]
        shards.append(o.transpose(0, 2, 1).reshape(nt * tile_n, D)[:ns])
    return np.concatenate(shards, axis=0)


def kernel(curr_emb, alpha, msg):
    from concourse.bass_utils import run_bass_kernel_spmd

    if "nc" not in _cache:
        _cache["nc"] = build_program()
    nc = _cache["nc"]
    in_maps = make_in_maps(curr_emb, alpha, msg)
    res = run_bass_kernel_spmd(nc, in_maps, list(range(CORES)))
    return gather_out([res.results[c]["out"] for c in range(CORES)])


# revision 3
# speedup vs baseline: 2.4975x; 2.4118x over previous
"""Trainium2 Bass kernel for nn_Aggregator_32959579030024.

Computes out[n, d] = curr_emb[n, 0, d] + sum_k alpha[n, k, 0] * msg[n, k, d]
for N=100000, K=32, D=128 (fp32), sharded over 8 NeuronCores on the node dim.

Math: per 128-node tile, SBUF partition p holds msg row 128*g + p of the tile
(g = 4-node group, 32 groups/tile); each group's 128 partitions are the
(node-in-group m, neighbor k) rows of 4 nodes. A block-diagonal alpha tile
[128, 4] per group (alpha[4g+m, k] at partition 32m+k, column m) is the moving
operand of a matmul whose stationary operand is the msg slice [128, 128]:

    psum[d, m] += sum_{p=(m,k)} msg[(m,k), d] * alphadiag[(m,k), m]
               =  sum_k alpha[node, k] * msg[node, k, d]

PSUM holds the tile transposed as [d, node]. DVE adds host-transposed curr
(currT [D, NSP]) during PSUM evacuation; the d-major result is DMA'd out and
the host transposes it back.

Precision/perf: fp32 matmuls on trn2 stream weights at ~38 Gelem/s — PE-bound.
The host instead splits msg exactly into bf16 hi + lo (same DMA bytes) and
alpha into a + b, and each group runs 3 bf16 matmuls accumulating in fp32
PSUM:   m*alpha ~= h*a + h*b + l*a    (dropped l*b term ~2^-18 relative).

DMA: all operands are host-permuted into per-tile layouts so every DMA is a
few large contiguous descriptors per partition (16KB/partition msg tiles);
the node dim is zero-padded to a multiple of 128 so all tiles are uniform.
"""

import numpy as np

N, K, D = 100000, 32, 128
CORES = 8
NS = N // CORES              # 12500 nodes per shard
TILE_N = 128                 # nodes per tile
NT = (NS + TILE_N - 1) // TILE_N   # 98 tiles (last one zero-padded)
NSP = NT * TILE_N            # 12544 padded nodes per shard
NG = TILE_N // 4             # 32 groups of 4 nodes per tile

_cache = {}


def build_program(nt=NT, msg_bufs=4):
    import concourse.bacc as bacc
    import concourse.mybir as mybir
    import concourse.tile as tile

    nc = bacc.Bacc("TRN2", target_bir_lowering=False, debug=False)
    f32 = mybir.dt.float32
    bf16 = mybir.dt.bfloat16
    nsp = nt * TILE_N
    # Host-permuted per-tile layouts (see make_in_maps):
    #   msgp[t, p, g, hl, d]  = bf16 hi/lo of msg row (128g + p) of tile t
    #   aldg[t, p, g, hl, m]  = block-diag alpha (nonzero iff p//32 == m)
    msgp = nc.dram_tensor("msgp", [nt, 128, NG, 2, D], bf16, kind="ExternalInput")
    aldg = nc.dram_tensor("aldg", [nt, 128, NG, 2, 4], bf16, kind="ExternalInput")
    currT = nc.dram_tensor("currT", [D, nsp], f32, kind="ExternalInput")
    out = nc.dram_tensor("out", [nt, D, TILE_N], f32, kind="ExternalOutput")

    with tile.TileContext(nc) as tc:
        with (
            tc.tile_pool(name="msgpool", bufs=msg_bufs) as msgpool,
            tc.tile_pool(name="smallp", bufs=msg_bufs) as smallp,
            tc.tile_pool(name="outp", bufs=msg_bufs) as outp,
            tc.tile_pool(name="psump", bufs=4, space="PSUM") as psump,
        ):
            for t in range(nt):
                msg_t = msgpool.tile([128, NG, 2, D], bf16, tag="msg")
                nc.sync.dma_start(msg_t[:], msgp[t])

                al_t = smallp.tile([128, NG, 2, 4], bf16, tag="al")
                nc.sync.dma_start(al_t[:], aldg[t])

                cur_t = smallp.tile([128, TILE_N], f32, tag="curr")
                nc.sync.dma_start(cur_t[:], currT[:, t * TILE_N:(t + 1) * TILE_N])

                ps = psump.tile([128, TILE_N], f32, tag="ps")
                for g in range(NG):
                    o = ps[:, 4 * g:4 * g + 4]
                    h = msg_t[:, g, 0, :]
                    lo = msg_t[:, g, 1, :]
                    a = al_t[:, g, 0, :]
                    b = al_t[:, g, 1, :]
                    nc.tensor.matmul(o, h, a, start=True, stop=False)
                    nc.tensor.matmul(o, h, b, start=False, stop=False)
                    nc.tensor.matmul(o, lo, a, start=False, stop=True)

                ot = outp.tile([128, TILE_N], f32, tag="out")
                nc.vector.tensor_add(ot[:], ps[:], cur_t[:])
                nc.sync.dma_start(out[t], ot[:])

    nc.compile()
    return nc


def _split_bf16(x):
    import ml_dtypes

    hi = x.astype(ml_dtypes.bfloat16)
    lo = (x - hi.astype(np.float32)).astype(ml_dtypes.bfloat16)
    return hi, lo


def make_in_maps(curr_emb, alpha, msg, ns=NS):
    import ml_dtypes

    bf16 = ml_dtypes.bfloat16
    curr_emb = np.asarray(curr_emb, dtype=np.float32)
    alpha = np.asarray(alpha, dtype=np.float32)
    msg = np.asarray(msg, dtype=np.float32)
    n = curr_emb.shape[0]
    cores = n // ns
    nt = (ns + TILE_N - 1) // TILE_N
    nsp = nt * TILE_N
    pad = nsp - ns
    in_maps = []
    for c in range(cores):
        sl = slice(c * ns, (c + 1) * ns)

        m = msg[sl].reshape(ns * K, D)
        if pad:
            m = np.concatenate([m, np.zeros((pad * K, D), np.float32)], axis=0)
        m_hi, m_lo = _split_bf16(m)
        # [nsp*K, D] rows -> [nt, g, p, D] -> [nt, p, g, D]; stack hi/lo on
        # a new axis before d so each partition's row is contiguous.
        m_hi = m_hi.reshape(nt, NG, 128, D).transpose(0, 2, 1, 3)
        m_lo = m_lo.reshape(nt, NG, 128, D).transpose(0, 2, 1, 3)
        msgp = np.ascontiguousarray(np.stack([m_hi, m_lo], axis=3))

        a = alpha[sl, :, 0]
        if pad:
            a = np.concatenate([a, np.zeros((pad, K), np.float32)], axis=0)
        a_hi, a_lo = _split_bf16(a)
        # aldg[t, 32m+k, g, hl, m] = a_hl[128t + 4g + m, k]
        aldg = np.zeros((nt, 4, K, NG, 2, 4), dtype=bf16)
        ah = a_hi.reshape(nt, NG, 4, K)
        al = a_lo.reshape(nt, NG, 4, K)
        for mm in range(4):
            aldg[:, mm, :, :, 0, mm] = ah[:, :, mm, :].transpose(0, 2, 1)
            aldg[:, mm, :, :, 1, mm] = al[:, :, mm, :].transpose(0, 2, 1)
        aldg = np.ascontiguousarray(aldg.reshape(nt, 128, NG, 2, 4))

        cur = curr_emb[sl, 0, :]
        if pad:
            cur = np.concatenate([cur, np.zeros((pad, D), np.float32)], axis=0)

        in_maps.append({
            "msgp": msgp,
            "aldg": aldg,
            "currT": np.ascontiguousarray(cur.T),
        })
    return in_maps


def gather_out(per_core_outs, ns=NS):
    shards = []
    for o in per_core_outs:
        nt = o.shape[0]
        # [nt, D, TILE_N] -> [nt, TILE_N, D] -> [nsp, D] -> [ns, D]
        shards.append(o.transpose(0, 2, 1).reshape(nt * TILE_N, D)[:ns])
    return np.concatenate(shards, axis=0)


def kernel(curr_emb, alpha, msg):
    from concourse.bass_utils import run_bass_kernel_spmd

    if "nc" not in _cache:
        _cache["nc"] = build_program()
    nc = _cache["nc"]
    in_maps = make_in_maps(curr_emb, alpha, msg)
    res = run_bass_kernel_spmd(nc, in_maps, list(range(CORES)))
    return gather_out([res.results[c]["out"] for c in range(CORES)])


# revision 5
# speedup vs baseline: 2.9097x; 1.1650x over previous
"""Trainium2 Bass kernel for nn_Aggregator_32959579030024.

Computes out[n, d] = curr_emb[n, 0, d] + sum_k alpha[n, k, 0] * msg[n, k, d]
for N=100000, K=32, D=128 (fp32), sharded over 8 NeuronCores on the node dim.

Math: per tile of `tile_n` nodes, SBUF partition p holds msg row 128*g + p of
the tile (g = 4-node group, tile_n/4 groups/tile); each group's 128 partitions
are the (node-in-group m, neighbor k) rows of 4 nodes. A block-diagonal alpha
tile [128, 4] per group (alpha[4g+m, k] at partition 32m+k, column m) is the
moving operand of a matmul whose stationary operand is the msg slice
[128, 128]:

    psum[d, m] += sum_{p=(m,k)} msg[(m,k), d] * alphadiag[(m,k), m]
               =  sum_k alpha[node, k] * msg[node, k, d]

PSUM holds the tile transposed as [d, node]. DVE adds host-transposed curr
during PSUM evacuation; the d-major result is DMA'd out and the host
transposes it back.

Precision/perf: fp32 matmuls on trn2 stream weights at ~38 Gelem/s (PE-bound),
so the host splits msg exactly into bf16 hi + lo (same DMA bytes) and alpha
into a + b; each group runs 3 bf16 matmuls accumulating in fp32 PSUM:
    m*alpha ~= h*a + h*b + l*a     (dropped l*b term ~2^-18 relative).

DMA: msg, block-diag alpha, and curr (bit-cast to bf16 pairs) are host-packed
into ONE contiguous per-tile block ([128 partitions, 17KB] for tile_n=128) so
each tile needs a single read DMA of full-size packets — measured pure-DMA
rate here is ~415 GB/s vs ~282 GB/s when small strided DMAs fragment the
queues. The node dim is zero-padded to a tile multiple so tiles are uniform.
"""

import numpy as np

N, K, D = 100000, 32, 128
CORES = 8
NS = N // CORES              # 12500 nodes per shard
TILE_N = 128                 # nodes per tile (kernel default)
MSG_BUFS = 6

_cache = {}


def _dims(ns, tile_n):
    nt = (ns + tile_n - 1) // tile_n
    ng = tile_n // 4
    mg = ng * 2 * D          # bf16 elems of msg hi/lo per partition
    ag = ng * 2 * 4          # bf16 elems of block-diag alpha per partition
    cg = 2 * tile_n          # bf16 elems (bit-cast fp32 curr) per partition
    return nt, ng, mg, ag, mg + ag + cg


def build_program(ns=NS, tile_n=TILE_N, msg_bufs=MSG_BUFS):
    import concourse.bacc as bacc
    import concourse.mybir as mybir
    import concourse.tile as tile

    nt, ng, mg, ag, F = _dims(ns, tile_n)
    nc = bacc.Bacc("TRN2", target_bir_lowering=False, debug=False)
    f32 = mybir.dt.float32
    bf16 = mybir.dt.bfloat16
    u16 = mybir.dt.uint16
    inp = nc.dram_tensor("inp", [nt, 128, F], u16, kind="ExternalInput")
    out = nc.dram_tensor("out", [nt, D, tile_n], f32, kind="ExternalOutput")

    with tile.TileContext(nc) as tc:
        with (
            tc.tile_pool(name="inpool", bufs=msg_bufs) as inpool,
            tc.tile_pool(name="outp", bufs=4) as outp,
            tc.tile_pool(name="psump", bufs=4, space="PSUM") as psump,
        ):
            for t in range(nt):
                it = inpool.tile([128, F], u16, tag="inp")
                nc.sync.dma_start(it[:], inp[t])
                msgv = it[:, :mg].bitcast(bf16).rearrange("p (g two d) -> p g two d", two=2, d=D)
                alv = it[:, mg:mg + ag].bitcast(bf16).rearrange(
                    "p (g two f) -> p g two f", two=2, f=4
                )
                curv = it[:, mg + ag:].bitcast(f32)

                ps = psump.tile([128, tile_n], f32, tag="ps")
                for g in range(ng):
                    o = ps[:, 4 * g:4 * g + 4]
                    h = msgv[:, g, 0, :]
                    lo = msgv[:, g, 1, :]
                    a = alv[:, g, 0, :]
                    b = alv[:, g, 1, :]
                    nc.tensor.matmul(o, h, a, start=True, stop=False)
                    nc.tensor.matmul(o, h, b, start=False, stop=False)
                    nc.tensor.matmul(o, lo, a, start=False, stop=True)

                ot = outp.tile([128, tile_n], f32, tag="out")
                nc.vector.tensor_add(ot[:], ps[:], curv[:])
                nc.sync.dma_start(out[t], ot[:])

    nc.compile()
    return nc


def _split_bf16(x):
    import ml_dtypes

    hi = x.astype(ml_dtypes.bfloat16)
    lo = (x - hi.astype(np.float32)).astype(ml_dtypes.bfloat16)
    return hi, lo


def make_in_maps(curr_emb, alpha, msg, ns=NS, tile_n=TILE_N):
    import ml_dtypes

    bf16 = ml_dtypes.bfloat16
    curr_emb = np.asarray(curr_emb, dtype=np.float32)
    alpha = np.asarray(alpha, dtype=np.float32)
    msg = np.asarray(msg, dtype=np.float32)
    n = curr_emb.shape[0]
    cores = n // ns
    nt, ng, mg, ag, F = _dims(ns, tile_n)
    nsp = nt * tile_n
    pad = nsp - ns
    in_maps = []
    for c in range(cores):
        sl = slice(c * ns, (c + 1) * ns)

        m = msg[sl].reshape(ns * K, D)
        if pad:
            m = np.concatenate([m, np.zeros((pad * K, D), np.float32)], axis=0)
        m_hi, m_lo = _split_bf16(m)
        # rows (128g + p) -> [nt, p, g, hl, d], flattened per partition
        m_hi = m_hi.reshape(nt, ng, 128, D).transpose(0, 2, 1, 3)
        m_lo = m_lo.reshape(nt, ng, 128, D).transpose(0, 2, 1, 3)
        msg_part = np.stack([m_hi, m_lo], axis=3).reshape(nt, 128, mg)

        a = alpha[sl, :, 0]
        if pad:
            a = np.concatenate([a, np.zeros((pad, K), np.float32)], axis=0)
        a_hi, a_lo = _split_bf16(a)
        # aldg[t, 32m+k, g, hl, m] = a_hl[t*tile_n + 4g + m, k]
        aldg = np.zeros((nt, 4, K, ng, 2, 4), dtype=bf16)
        ah = a_hi.reshape(nt, ng, 4, K)
        al = a_lo.reshape(nt, ng, 4, K)
        for mm in range(4):
            aldg[:, mm, :, :, 0, mm] = ah[:, :, mm, :].transpose(0, 2, 1)
            aldg[:, mm, :, :, 1, mm] = al[:, :, mm, :].transpose(0, 2, 1)
        al_part = aldg.reshape(nt, 128, ag)

        cur = curr_emb[sl, 0, :]
        if pad:
            cur = np.concatenate([cur, np.zeros((pad, D), np.float32)], axis=0)
        # currT[d, tile nodes] bit-cast to bf16 pairs: [nt, 128(d), 2*tile_n]
        curT = np.ascontiguousarray(cur.T)  # [D, nsp]
        cur_part = (
            curT.reshape(D, nt, tile_n).transpose(1, 0, 2)
            .copy().view(bf16).reshape(nt, 128, 2 * tile_n)
        )

        combined = np.concatenate(
            [msg_part.view(np.uint16), al_part.view(np.uint16),
             cur_part.view(np.uint16)], axis=2
        )
        in_maps.append({"inp": np.ascontiguousarray(combined)})
    return in_maps


def gather_out(per_core_outs, ns=NS, tile_n=TILE_N):
    shards = []
    for o in per_core_outs:
        nt = o.shape[0]
        # [nt, D, tile_n] -> [nt, tile_n, D] -> [nsp, D] -> [ns, D]
        shards.append(o.transpose(0, 2, 1).reshape(nt * tile_n, D)[:ns])
    return np.concatenate(shards, axis=0)


def kernel(curr_emb, alpha, msg):
    from concourse.bass_utils import run_bass_kernel_spmd

    if "nc" not in _cache:
        _cache["nc"] = build_program()
    nc = _cache["nc"]
    in_maps = make_in_maps(curr_emb, alpha, msg)
    res = run_bass_kernel_spmd(nc, in_maps, list(range(CORES)))
    return gather_out([res.results[c]["out"] for c in range(CORES)])


# revision 8
# speedup vs baseline: 3.1278x; 1.0750x over previous
"""Trainium2 Bass kernel for nn_Aggregator_32959579030024.

Computes out[n, d] = curr_emb[n, 0, d] + sum_k alpha[n, k, 0] * msg[n, k, d]
for N=100000, K=32, D=128 (fp32), sharded over 8 NeuronCores on the node dim.

Math: per tile of `tile_n` nodes, SBUF partition p holds msg row 128*g + p of
the tile (g = 4-node group, tile_n/4 groups/tile); each group's 128 partitions
are the (node-in-group m, neighbor k) rows of 4 nodes. A block-diagonal alpha
tile [128, 4] per group (alpha[4g+m, k] at partition 32m+k, column m) is the
moving operand of a matmul whose stationary operand is the msg slice
[128, 128]:

    psum[d, m] += sum_{p=(m,k)} msg[(m,k), d] * alphadiag[(m,k), m]
               =  sum_k alpha[node, k] * msg[node, k, d]

PSUM holds the tile transposed as [d, node]. DVE adds host-transposed curr
during PSUM evacuation; the d-major result is DMA'd out and the host
transposes it back.

Precision/perf: fp32 matmuls on trn2 stream weights at ~38 Gelem/s (PE-bound),
so the host splits msg exactly into bf16 hi + lo (same DMA bytes) and alpha
into a + b; each group runs 3 bf16 matmuls accumulating in fp32 PSUM:
    m*alpha ~= h*a + h*b + l*a     (dropped l*b term ~2^-18 relative).

DMA: msg, block-diag alpha, and curr (bit-cast to bf16 pairs) are host-packed
into ONE contiguous per-tile block ([128 partitions, 17KB] for tile_n=128) so
each tile needs a single read DMA of full-size packets — measured pure-DMA
rate here is ~415 GB/s vs ~282 GB/s when small strided DMAs fragment the
queues. The node dim is zero-padded to a tile multiple so tiles are uniform.
"""

import numpy as np

N, K, D = 100000, 32, 128
CORES = 8
NS = N // CORES              # 12500 nodes per shard
TILE_N = 256                 # nodes per tile (kernel default)
MSG_BUFS = 4
OUT_BATCH = 7                # tiles per batched output DMA

_cache = {}


def _dims(ns, tile_n):
    nt = (ns + tile_n - 1) // tile_n
    ng = tile_n // 4
    mg = ng * 2 * D          # bf16 elems of msg hi/lo per partition
    ag = ng * 4 * 4          # bf16 elems of block-diag alpha per partition
    cg = 2 * tile_n          # bf16 elems (bit-cast fp32 curr) per partition
    return nt, ng, mg, ag, mg + ag + cg


def build_program(ns=NS, tile_n=TILE_N, msg_bufs=MSG_BUFS, ob=OUT_BATCH):
    import concourse.bacc as bacc
    import concourse.mybir as mybir
    import concourse.tile as tile

    nt, ng, mg, ag, F = _dims(ns, tile_n)
    nc = bacc.Bacc("TRN2", target_bir_lowering=False, debug=False)
    f32 = mybir.dt.float32
    bf16 = mybir.dt.bfloat16
    u16 = mybir.dt.uint16
    inp = nc.dram_tensor("inp", [nt, 128, F], u16, kind="ExternalInput")
    assert nt % ob == 0, (nt, ob)
    out = nc.dram_tensor("out", [nt // ob, D, ob * tile_n], f32, kind="ExternalOutput")

    with tile.TileContext(nc) as tc:
        with (
            tc.tile_pool(name="inpool", bufs=msg_bufs) as inpool,
            tc.tile_pool(name="outp", bufs=4) as outp,
            tc.tile_pool(name="psump", bufs=4, space="PSUM") as psump,
        ):
            for t in range(nt):
                it = inpool.tile([128, F], u16, tag="inp")
                nc.sync.dma_start(it[:], inp[t])
                msgv = it[:, :mg].bitcast(bf16).rearrange("p (g two d) -> p g two d", two=2, d=D)
                alv = it[:, mg:mg + ag].bitcast(bf16).rearrange(
                    "p (g four f) -> p g four f", four=4, f=4
                )
                curv = it[:, mg + ag:].bitcast(f32)

                # psum holds [d, g, hl, m]: hl=0 accumulates h*a + l*a,
                # hl=1 holds h*b; the two halves are summed during evac.
                ps = psump.tile([128, ng, 2, 4], f32, tag="ps")
                for g in range(ng):
                    h = msgv[:, g, 0, :]
                    lo = msgv[:, g, 1, :]
                    ab = alv[:, g, 0:2, :]     # [a | b]
                    az = alv[:, g, 2:4, :]     # [a | 0]
                    nc.tensor.matmul(ps[:, g, :, :], h, ab, start=True, stop=False)
                    nc.tensor.matmul(ps[:, g, :, :], lo, az, start=False, stop=True)

                if t % ob == 0:
                    ot = outp.tile([128, ob * tile_n], f32, tag="out")
                osl = ot[:, (t % ob) * tile_n:(t % ob + 1) * tile_n].rearrange(
                    "p (g m) -> p g m", m=4
                )
                cur3 = curv.rearrange("p (g m) -> p g m", m=4)
                nc.vector.tensor_add(osl, ps[:, :, 0, :], cur3)
                nc.vector.tensor_add(osl, osl, ps[:, :, 1, :])
                if t % ob == ob - 1:
                    nc.sync.dma_start(out[t // ob], ot[:])

    nc.compile()
    return nc


def _split_bf16(x):
    import ml_dtypes

    hi = x.astype(ml_dtypes.bfloat16)
    lo = (x - hi.astype(np.float32)).astype(ml_dtypes.bfloat16)
    return hi, lo


def make_in_maps(curr_emb, alpha, msg, ns=NS, tile_n=TILE_N):
    import ml_dtypes

    bf16 = ml_dtypes.bfloat16
    curr_emb = np.asarray(curr_emb, dtype=np.float32)
    alpha = np.asarray(alpha, dtype=np.float32)
    msg = np.asarray(msg, dtype=np.float32)
    n = curr_emb.shape[0]
    cores = n // ns
    nt, ng, mg, ag, F = _dims(ns, tile_n)
    nsp = nt * tile_n
    pad = nsp - ns
    in_maps = []
    for c in range(cores):
        sl = slice(c * ns, (c + 1) * ns)

        m = msg[sl].reshape(ns * K, D)
        if pad:
            m = np.concatenate([m, np.zeros((pad * K, D), np.float32)], axis=0)
        m_hi, m_lo = _split_bf16(m)
        # rows (128g + p) -> [nt, p, g, hl, d], flattened per partition
        m_hi = m_hi.reshape(nt, ng, 128, D).transpose(0, 2, 1, 3)
        m_lo = m_lo.reshape(nt, ng, 128, D).transpose(0, 2, 1, 3)
        msg_part = np.stack([m_hi, m_lo], axis=3).reshape(nt, 128, mg)

        a = alpha[sl, :, 0]
        if pad:
            a = np.concatenate([a, np.zeros((pad, K), np.float32)], axis=0)
        a_hi, a_lo = _split_bf16(a)
        # aldg[t, 32m+k, g, q, m]: quads q = (a, b, a, 0) of block-diag alpha
        aldg = np.zeros((nt, 4, K, ng, 4, 4), dtype=bf16)
        ah = a_hi.reshape(nt, ng, 4, K)
        al = a_lo.reshape(nt, ng, 4, K)
        for mm in range(4):
            aht = ah[:, :, mm, :].transpose(0, 2, 1)
            aldg[:, mm, :, :, 0, mm] = aht
            aldg[:, mm, :, :, 1, mm] = al[:, :, mm, :].transpose(0, 2, 1)
            aldg[:, mm, :, :, 2, mm] = aht
        al_part = aldg.reshape(nt, 128, ag)

        cur = curr_emb[sl, 0, :]
        if pad:
            cur = np.concatenate([cur, np.zeros((pad, D), np.float32)], axis=0)
        # currT[d, tile nodes] bit-cast to bf16 pairs: [nt, 128(d), 2*tile_n]
        curT = np.ascontiguousarray(cur.T)  # [D, nsp]
        cur_part = (
            curT.reshape(D, nt, tile_n).transpose(1, 0, 2)
            .copy().view(bf16).reshape(nt, 128, 2 * tile_n)
        )

        combined = np.concatenate(
            [msg_part.view(np.uint16), al_part.view(np.uint16),
             cur_part.view(np.uint16)], axis=2
        )
        in_maps.append({"inp": np.ascontiguousarray(combined)})
    return in_maps


def gather_out(per_core_outs, ns=NS, tile_n=TILE_N):
    shards = []
    for o in per_core_outs:
        nb = o.shape[0] * o.shape[2]  # total padded nodes
        # [ntg, D, ob*tile_n] -> [ntg, ob*tile_n, D] -> [nsp, D] -> [ns, D]
        shards.append(o.transpose(0, 2, 1).reshape(nb, D)[:ns])
    return np.concatenate(shards, axis=0)


def kernel(curr_emb, alpha, msg):
    from concourse.bass_utils import run_bass_kernel_spmd

    if "nc" not in _cache:
        _cache["nc"] = build_program()
    nc = _cache["nc"]
    in_maps = make_in_maps(curr_emb, alpha, msg)
    res = run_bass_kernel_spmd(nc, in_maps, list(range(CORES)))
    return gather_out([res.results[c]["out"] for c in range(CORES)])


# revision 10
# speedup vs baseline: 3.4583x; 1.1057x over previous
"""Trainium2 Bass kernel for nn_Aggregator_32959579030024.

Computes out[n, d] = curr_emb[n, 0, d] + sum_k alpha[n, k, 0] * msg[n, k, d]
for N=100000, K=32, D=128 (fp32), sharded over 8 NeuronCores on the node dim.

Math: per tile of `tile_n` nodes, SBUF partition p holds msg row 128*g + p of
the tile (g = 4-node group, tile_n/4 groups/tile); each group's 128 partitions
are the (node-in-group m, neighbor k) rows of 4 nodes. A block-diagonal alpha
tile [128, 4] per group (alpha[4g+m, k] at partition 32m+k, column m) is the
moving operand of a matmul whose stationary operand is the msg slice
[128, 128]:

    psum[d, m] += sum_{p=(m,k)} msg[(m,k), d] * alphadiag[(m,k), m]
               =  sum_k alpha[node, k] * msg[node, k, d]

PSUM holds the tile transposed as [d, node]. DVE adds host-transposed curr
during PSUM evacuation; the d-major result is DMA'd out and the host
transposes it back.

Precision/perf: fp32 matmuls on trn2 stream weights at ~38 Gelem/s (PE-bound),
so the host splits msg exactly into bf16 hi + lo (same DMA bytes) and alpha
into a + b; each group runs 3 bf16 matmuls accumulating in fp32 PSUM:
    m*alpha ~= h*a + h*b + l*a     (dropped l*b term ~2^-18 relative).

DMA: msg, block-diag alpha, and curr (bit-cast to bf16 pairs) are host-packed
into ONE contiguous per-tile block ([128 partitions, 17KB] for tile_n=128) so
each tile needs a single read DMA of full-size packets — measured pure-DMA
rate here is ~415 GB/s vs ~282 GB/s when small strided DMAs fragment the
queues. The node dim is zero-padded to a tile multiple so tiles are uniform.
"""

import numpy as np

N, K, D = 100000, 32, 128
CORES = 8
NS = N // CORES              # 12500 nodes per shard
TILE_N = 256                 # nodes per tile (kernel default)
MSG_BUFS = 4
OUT_BATCH = 7                # tiles per batched output DMA

_cache = {}


def _dims(ns, tile_n):
    nt = (ns + tile_n - 1) // tile_n
    ng = tile_n // 4
    mg = ng * 2 * D          # bf16 elems of msg hi/lo per partition
    ag = ng * 3              # bf16 elems of compact (a,b,a) alpha per partition
    cg = 2 * tile_n          # bf16 elems (bit-cast fp32 curr) per partition
    return nt, ng, mg, ag, mg + ag + cg


def build_program(ns=NS, tile_n=TILE_N, msg_bufs=MSG_BUFS, ob=OUT_BATCH):
    import concourse.bacc as bacc
    import concourse.mybir as mybir
    import concourse.tile as tile

    nt, ng, mg, ag, F = _dims(ns, tile_n)
    nc = bacc.Bacc("TRN2", target_bir_lowering=False, debug=False)
    f32 = mybir.dt.float32
    bf16 = mybir.dt.bfloat16
    u16 = mybir.dt.uint16
    inp = nc.dram_tensor("inp", [nt, 128, F], u16, kind="ExternalInput")
    assert nt % ob == 0, (nt, ob)
    out = nc.dram_tensor("out", [nt // ob, D, ob * tile_n], f32, kind="ExternalOutput")

    with tile.TileContext(nc) as tc:
        with (
            tc.tile_pool(name="inpool", bufs=msg_bufs) as inpool,
            tc.tile_pool(name="alpool", bufs=1) as alpool,
            tc.tile_pool(name="outp", bufs=4) as outp,
            tc.tile_pool(name="psump", bufs=4, space="PSUM") as psump,
        ):
            # Persistent block-diag alpha buffers: zeroed once; each tile
            # rewrites only the (fixed) diagonal slots, so off-diagonal
            # zeros and the fourth (zero) quad survive across tiles.
            AB = 3
            al_bufs = [
                alpool.tile([128, ng, 4, 4], bf16, name=f"albuf{i}",
                            tag=f"al{i}")
                for i in range(AB)
            ]
            for ab in al_bufs:
                nc.vector.memset(ab[:], 0.0)
            for t in range(nt):
                it = inpool.tile([128, F], u16, tag="inp")
                nc.sync.dma_start(it[:], inp[t])
                msgv = it[:, :mg].bitcast(bf16).rearrange("p (g two d) -> p g two d", two=2, d=D)
                acv = it[:, mg:mg + ag].bitcast(bf16).rearrange(
                    "p (g three) -> p g three", three=3
                )
                curv = it[:, mg + ag:].bitcast(f32)

                al_t = al_bufs[t % AB]
                for m in range(4):
                    nc.vector.tensor_copy(
                        al_t[32 * m:32 * (m + 1), :, 0:3, m],
                        acv[32 * m:32 * (m + 1), :, :],
                    )

                # psum holds [d, g, hl, m]: hl=0 accumulates h*a + l*a,
                # hl=1 holds h*b; the two halves are summed during evac.
                ps = psump.tile([128, ng, 2, 4], f32, tag="ps")
                for g in range(ng):
                    h = msgv[:, g, 0, :]
                    lo = msgv[:, g, 1, :]
                    ab = al_t[:, g, 0:2, :]    # [a | b]
                    az = al_t[:, g, 2:4, :]    # [a | 0]
                    nc.tensor.matmul(ps[:, g, :, :], h, ab, start=True, stop=False)
                    nc.tensor.matmul(ps[:, g, :, :], lo, az, start=False, stop=True)

                if t % ob == 0:
                    ot = outp.tile([128, ob * tile_n], f32, tag="out")
                osl = ot[:, (t % ob) * tile_n:(t % ob + 1) * tile_n].rearrange(
                    "p (g m) -> p g m", m=4
                )
                cur3 = curv.rearrange("p (g m) -> p g m", m=4)
                nc.vector.tensor_add(osl, ps[:, :, 0, :], cur3)
                nc.vector.tensor_add(osl, osl, ps[:, :, 1, :])
                if t % ob == ob - 1:
                    nc.sync.dma_start(out[t // ob], ot[:])

    nc.compile()
    return nc


def _split_bf16(x):
    import ml_dtypes

    hi = x.astype(ml_dtypes.bfloat16)
    lo = (x - hi.astype(np.float32)).astype(ml_dtypes.bfloat16)
    return hi, lo


def make_in_maps(curr_emb, alpha, msg, ns=NS, tile_n=TILE_N):
    import ml_dtypes

    bf16 = ml_dtypes.bfloat16
    curr_emb = np.asarray(curr_emb, dtype=np.float32)
    alpha = np.asarray(alpha, dtype=np.float32)
    msg = np.asarray(msg, dtype=np.float32)
    n = curr_emb.shape[0]
    cores = n // ns
    nt, ng, mg, ag, F = _dims(ns, tile_n)
    nsp = nt * tile_n
    pad = nsp - ns
    in_maps = []
    for c in range(cores):
        sl = slice(c * ns, (c + 1) * ns)

        m = msg[sl].reshape(ns * K, D)
        if pad:
            m = np.concatenate([m, np.zeros((pad * K, D), np.float32)], axis=0)
        m_hi, m_lo = _split_bf16(m)
        # rows (128g + p) -> [nt, p, g, hl, d], flattened per partition
        m_hi = m_hi.reshape(nt, ng, 128, D).transpose(0, 2, 1, 3)
        m_lo = m_lo.reshape(nt, ng, 128, D).transpose(0, 2, 1, 3)
        msg_part = np.stack([m_hi, m_lo], axis=3).reshape(nt, 128, mg)

        a = alpha[sl, :, 0]
        if pad:
            a = np.concatenate([a, np.zeros((pad, K), np.float32)], axis=0)
        a_hi, a_lo = _split_bf16(a)
        # Compact (a, b, a) per diag slot: aldg[t, 32m+k, g, q] = alpha
        # quads for node 4g+m, neighbor k (expanded to block-diag on-chip).
        aldg = np.zeros((nt, 4, K, ng, 3), dtype=bf16)
        ah = a_hi.reshape(nt, ng, 4, K)
        al = a_lo.reshape(nt, ng, 4, K)
        for mm in range(4):
            aht = ah[:, :, mm, :].transpose(0, 2, 1)
            aldg[:, mm, :, :, 0] = aht
            aldg[:, mm, :, :, 1] = al[:, :, mm, :].transpose(0, 2, 1)
            aldg[:, mm, :, :, 2] = aht
        al_part = aldg.reshape(nt, 128, ag)

        cur = curr_emb[sl, 0, :]
        if pad:
            cur = np.concatenate([cur, np.zeros((pad, D), np.float32)], axis=0)
        # currT[d, tile nodes] bit-cast to bf16 pairs: [nt, 128(d), 2*tile_n]
        curT = np.ascontiguousarray(cur.T)  # [D, nsp]
        cur_part = (
            curT.reshape(D, nt, tile_n).transpose(1, 0, 2)
            .copy().view(bf16).reshape(nt, 128, 2 * tile_n)
        )

        combined = np.concatenate(
            [msg_part.view(np.uint16), al_part.view(np.uint16),
             cur_part.view(np.uint16)], axis=2
        )
        in_maps.append({"inp": np.ascontiguousarray(combined)})
    return in_maps


def gather_out(per_core_outs, ns=NS, tile_n=TILE_N):
    shards = []
    for o in per_core_outs:
        nb = o.shape[0] * o.shape[2]  # total padded nodes
        # [ntg, D, ob*tile_n] -> [ntg, ob*tile_n, D] -> [nsp, D] -> [ns, D]
        shards.append(o.transpose(0, 2, 1).reshape(nb, D)[:ns])
    return np.concatenate(shards, axis=0)


def kernel(curr_emb, alpha, msg):
    from concourse.bass_utils import run_bass_kernel_spmd

    if "nc" not in _cache:
        _cache["nc"] = build_program()
    nc = _cache["nc"]
    in_maps = make_in_maps(curr_emb, alpha, msg)
    res = run_bass_kernel_spmd(nc, in_maps, list(range(CORES)))
    return gather_out([res.results[c]["out"] for c in range(CORES)])
